# revision 33
# baseline (speedup 1.0000x reference)
"""Trainium2 Bass kernel for nn_Interpolator — Gaussian-scatter + P2P, v4.

Reference (N=32768 obs, R=2048 sorted ref timesteps, ninp=64, a=50):
    Ks[r,n] = exp(-a(ref_r - t_n)^2)*mask + EPS,  Kc same with 10a
    lam_s = Ks@onehot + EPS, num_s = Ks@(onehot*v), likewise coarse
    lam = lam_s/R; cross = (num_s@rho)/rowsum(lam_s); coarse = num_c/lam_c
    out = concat([lam, cross, coarse-cross], -1)   [1, R, 192]

Algorithm (NUFFT-style Gaussian gridding): scatter each observation onto a
uniform G=128 grid with a narrow Gaussian V (sigma_v = 1.5*dg), so
B_T[h,q] = sum_n V(h - t_n) * comb[n,q] accumulates ALL four segment sums
in one [128,128,128] matmul per 128-obs chunk (comb = [onehot*mask |
onehot*mask*v]).  Both kernels are then recovered exactly by grid-to-ref
matmuls with host-precomputed deconvolved Gaussians Kp (Gaussian*Gaussian
convolution identity; aliasing error ~e^-37).  No transposes needed:
the loop matmul directly produces the [h, q] layout the finish consumes.

Obs axis sharded 8 ways.  The partial B_T [128,128] bf16 (32 KB) is
combined across cores with a CC AllReduce (CCE add in-fabric, Shared
output) — the 32 KB reduced result gathers back in one 128-descriptor
DMA and IS the finish input, beating AllGather + local tree-reduce whose
rank-major 256 KB gather cost 1024 descriptors.  The one-time ncfw
first-call wakeup (~20-55 us trigger-to-mesh, run-variant) fully hides
the main loop and constant DMAs.  (A remote_dma_broadcast P2P exchange
was tried and rejected: the SWDGE path services ~10 us/descriptor and a
128-partition SBUF payload needs >=128 descriptors per destination.)
Each core then finishes only its own 256 ref columns: grid->ref matmuls,
EPS corrections (the cross-denominator D comes from a rowsum of bred
projected through Kp_s, its EPS part being the host constant
EPS*(N+NI)), reciprocals, bf16 rho matmul, and a single merged
[64, 3, 256] output tile shipped with two DMAs; the host reassembles the
[R, 192] result without a transpose.
"""

import os
import sys

import numpy as np

sys.path.insert(0, "/opt/trn_rl_repo")

import concourse.bass as bass
import concourse.tile as tile
from concourse import bacc, mybir

# The image's antenv package lacks axon_hooks (NTFF profiling registry);
# register one so trace=True can profile HW exec time. Harmless if unused.
try:
    import antenv.axon_hooks  # noqa: F401
except ImportError:
    import types as _types

    _m = _types.ModuleType("antenv.axon_hooks")
    _m._hook = None

    def _set_hook(hook):
        _m._hook = hook

    def _get_hook():
        if _m._hook is None:
            try:
                from trn_agent_boot.trn_boot import _ntff_profile_via_ctypes

                _m._hook = _ntff_profile_via_ctypes("/opt/axon/libaxon_pjrt.so")
            except Exception:
                _m._hook = None
        return _m._hook

    _m.set_axon_ntff_profile_hook = _set_hook
    _m.get_axon_ntff_profile_hook = _get_hook
    sys.modules["antenv.axon_hooks"] = _m
    try:
        import antenv

        antenv.axon_hooks = _m
    except ImportError:
        pass

F32 = mybir.dt.float32
BF16 = mybir.dt.bfloat16
Alu = mybir.AluOpType
Act = mybir.ActivationFunctionType

N = 32768
R = 2048
NI = 64
M = 8
ND = N // M          # 4096 obs per core
P = 128
NCHUNK = ND // P     # 32
G = 128              # scatter grid points
RS = R // M          # 256 ref rows finished per core
EPS = 1e-7
K_SCALE = 10.0

GRID_LO = -0.05
GRID_HI = 1.05
DG = (GRID_HI - GRID_LO) / (G - 1)
SIG_V = 1.5 * DG
BV = 1.0 / (2.0 * SIG_V * SIG_V)


def build_program():
    nc = bacc.Bacc("TRN2")

    # host-reordered so every DMA is contiguous: comb[p, c, :] = row c*128+p
    comb_in = nc.declare_dram_parameter(
        "comb", [P, NCHUNK, 2 * NI], BF16, isOutput=False
    )
    gb_in = nc.declare_dram_parameter("gb", [P, G], F32, isOutput=False)
    tsc_in = nc.declare_dram_parameter("tsc", [P, NCHUNK], F32, isOutput=False)
    rho_in = nc.declare_dram_parameter("rho", [NI, NI], F32, isOutput=False)
    # corr[0:64] = EPS*(cnt+1); corr[64:128] = EPS*sv  (per-dim EPS pads)
    corr_in = nc.declare_dram_parameter("corr", [P, 1], F32, isOutput=False)
    # per-core deconvolved grid->ref kernels: [G, {smooth,coarse}, RS]
    kp_in = nc.declare_dram_parameter("kp", [G, 2, RS], BF16, isOutput=False)
    # output slice [q, {lam,cross,transient}, r]; host reassembles [RS, 192]
    out_t = nc.declare_dram_parameter("out", [NI, 3, RS], F32, isOutput=True)

    groups = [list(range(M))]

    with tile.TileContext(nc) as tc:
        with (
            tc.tile_pool(name="consts", bufs=1) as consts,
            tc.tile_pool(name="dram", bufs=1, space="DRAM") as dram,
        ):
            # ---------------- constants ----------------
            gb = consts.tile([P, G], F32)
            nc.sync.dma_start(out=gb[:], in_=gb_in[:])
            tsc = consts.tile([P, NCHUNK], F32)
            nc.sync.dma_start(out=tsc[:], in_=tsc_in[:])
            combH = consts.tile([P, NCHUNK, 2 * NI], BF16)
            for q4 in range(4):
                cs = q4 * (NCHUNK // 4)
                ce = cs + NCHUNK // 4
                nc.sync.dma_start(
                    out=combH[:, cs:ce, :], in_=comb_in[:, cs:ce, :]
                )
            corr_col = consts.tile([P, 1], F32)
            nc.sync.dma_start(out=corr_col[:], in_=corr_in[:])
            rho_sb = consts.tile([NI, NI], F32)
            nc.sync.dma_start(out=rho_sb[:], in_=rho_in[:])
            kp_sb = consts.tile([G, 2, RS], BF16)
            nc.sync.dma_start(out=kp_sb[:], in_=kp_in[:])
            ones_row_bf = consts.tile([1, P], BF16)
            nc.vector.memset(ones_row_bf, 1.0)
            rho_bf = consts.tile([NI, NI], BF16)
            nc.vector.tensor_copy(out=rho_bf[:], in_=rho_sb[:])

            part = consts.tile([P, G], BF16)

            # ---------------- main loop ----------------
            with (
                tc.tile_pool(name="acc", bufs=1, space="PSUM") as accpool,
                tc.tile_pool(name="work", bufs=4) as work,
            ):
                acc = accpool.tile([P, G], F32, name="acc", tag="acc")

                for c in range(NCHUNK):
                    d = work.tile([P, G], F32, tag="d")
                    nc.vector.tensor_scalar(
                        out=d[:], in0=gb[:], scalar1=tsc[:, c : c + 1],
                        scalar2=None, op0=Alu.subtract,
                    )
                    d2 = work.tile([P, G], F32, tag="d2")
                    nc.gpsimd.tensor_tensor(
                        out=d2[:], in0=d[:], in1=d[:], op=Alu.mult
                    )
                    kek = work.tile([P, G], BF16, tag="kek")
                    nc.scalar.activation(
                        out=kek[:], in_=d2[:], func=Act.Exp, scale=-BV
                    )
                    nc.tensor.matmul(
                        acc[:, :],
                        kek[:, :],
                        combH[:, c, :],
                        start=(c == 0),
                        stop=(c == NCHUNK - 1),
                    )

                nc.vector.tensor_copy(out=part[:], in_=acc[:])

            # ---------------- AllReduce (CCE add in-fabric) ----------------
            # vs AllGather: same first-call mesh pacing, but the 32 KB Shared
            # output IS the reduced bred — kills the 256 KB rank-major gather
            # (1024 descriptors) and the 3-step local tree-reduce
            ar_in = dram.tile([P, G], BF16, name="ar_in")
            ar_out = dram.tile([P, G], BF16, name="ar_out", addr_space="Shared")
            nc.sync.dma_start(out=ar_in[:], in_=part[:])
            nc.gpsimd.collective_compute(
                "AllReduce",
                Alu.add,
                replica_groups=groups,
                ins=[ar_in[:].opt()],
                outs=[ar_out[:].opt()],
            )

            with (
                tc.tile_pool(name="fin", bufs=1) as fin,
                tc.tile_pool(name="fps", bufs=1, space="PSUM") as fps,
            ):
                bred = fin.tile([P, G], BF16)
                nc.sync.dma_start(out=bred[:], in_=ar_out[:])

                # D[r] = sum_q lam_s[q,r] folded to grid space: rowsum of
                # bred's lam-half -> one bf16 PE matmul (concurrent with the
                # interp matmuls); the EPS-corr part of D is the host
                # constant EPS*(N+NI)
                bsum = fin.tile([P, 1], F32)
                nc.vector.reduce_sum(
                    out=bsum[:], in_=bred[:, 0:NI],
                    axis=mybir.AxisListType.X, op=Alu.add,
                )
                bsum_bf = fin.tile([P, 1], BF16)
                nc.scalar.copy(out=bsum_bf[:], in_=bsum[:])

                # four [64, RS] grid->ref matmuls; bred free-sliced so every
                # PE output sits at partition base 0
                interp = {}
                for nm, q, qsl in [
                    ("ns", 0, slice(NI, P)),
                    ("lc", 1, slice(0, NI)),
                    ("nq", 1, slice(NI, P)),
                    ("ls", 0, slice(0, NI)),
                ]:
                    ip = fps.tile([NI, RS], F32, tag=f"ip_{nm}")
                    nc.tensor.matmul(
                        ip[:], bred[:, qsl], kp_sb[:, q, :],
                        start=True, stop=True,
                    )
                    dt_out = BF16 if nm == "ns" else F32
                    sb = fin.tile([NI, RS], dt_out, name=f"sb_{nm}")
                    cc = (
                        corr_col[0:NI, :]
                        if nm in ("ls", "lc")
                        else corr_col[NI:P, :]
                    )
                    if nm in ("ls", "lc"):
                        nc.vector.tensor_scalar(
                            out=sb[:], in0=ip[:], scalar1=cc, scalar2=None,
                            op0=Alu.add,
                        )
                    else:
                        # ACT engine: out = Identity(1.0*in + bias)
                        nc.scalar.activation(
                            out=sb[:], in_=ip[:], func=Act.Identity,
                            scale=1.0, bias=cc,
                        )
                    interp[nm] = sb

                ls, ns, lc, nq = (
                    interp[k] for k in ("ls", "ns", "lc", "nq")
                )

                # 1/D via the bsum projection (+ host-constant EPS corr)
                dps2 = fps.tile([1, RS], F32, tag="dps")
                nc.tensor.matmul(
                    dps2[:], bsum_bf[:], kp_sb[:, 0, :], start=True, stop=True
                )
                dsum = fin.tile([1, RS], F32)
                nc.vector.tensor_scalar(
                    out=dsum[:], in0=dps2[:], scalar1=EPS * (N + NI),
                    scalar2=None, op0=Alu.add,
                )
                recd = fin.tile([1, RS], F32)
                nc.vector.reciprocal_approx_fast(out=recd[:], in_=dsum[:])
                recd_bf = fin.tile([1, RS], BF16)
                nc.scalar.copy(out=recd_bf[:], in_=recd[:])
                dbp = fps.tile([NI, RS], F32, tag="dbp")
                nc.tensor.matmul(
                    dbp[:], ones_row_bf[0:1, 0:NI], recd_bf[0:1, :],
                    start=True, stop=True,
                )

                # all three output quantities land in one tile; two DMAs so
                # lam+cross ship while transient computes
                outb = fin.tile([NI, 3, RS], F32)

                rec = fin.tile([NI, RS], F32)
                nc.vector.reciprocal_approx_fast(out=rec[:], in_=lc[:])
                coarse = fin.tile([NI, RS], F32)
                nc.vector.tensor_mul(out=coarse[:], in0=nq[:], in1=rec[:])
                nc.vector.tensor_scalar(
                    out=outb[:, 0, :], in0=ls[:], scalar1=1.0 / R,
                    scalar2=None, op0=Alu.mult,
                )

                crp = fps.tile([NI, RS], F32, tag="crp")
                nc.tensor.matmul(crp[:], rho_bf[:], ns[:], start=True, stop=True)
                crp_sb = fin.tile([NI, RS], F32)
                nc.scalar.copy(out=crp_sb[:], in_=crp[:])
                # cross: one PSUM operand (dbp) + SBUF crp copy — no dbc hop
                nc.vector.tensor_mul(
                    out=outb[:, 1, :], in0=dbp[:], in1=crp_sb[:]
                )
                nc.sync.dma_start(out=out_t[:, 0:2, :], in_=outb[:, 0:2, :])
                nc.vector.tensor_sub(
                    out=outb[:, 2, :], in0=coarse[:], in1=outb[:, 1, :]
                )
                nc.sync.dma_start(out=out_t[:, 2, :], in_=outb[:, 2, :])

    nc.finalize()
    return nc


_prog_cache = {}


def _get_prog():
    if "p" not in _prog_cache:
        _prog_cache["p"] = build_program()
    return _prog_cache["p"]


last_results = None


def kernel(S, reference_timesteps, alpha, rho):
    global last_results
    import ml_dtypes

    S = np.ascontiguousarray(np.asarray(S, dtype=np.float32))
    ref = np.ascontiguousarray(
        np.asarray(reference_timesteps, dtype=np.float32)
    )
    rho = np.ascontiguousarray(np.asarray(rho, dtype=np.float32))
    a = float(np.asarray(alpha).reshape(-1)[0])

    assert S.shape == (N, 3) and ref.shape == (1, R) and rho.shape == (NI, NI)

    refd = ref[0].astype(np.float64)
    grid = GRID_LO + DG * np.arange(G)

    # deconvolved grid->ref kernels (Gaussian convolution identity)
    sig_s = 1.0 / np.sqrt(2.0 * a)
    sig_c = 1.0 / np.sqrt(2.0 * K_SCALE * a)

    def kp_mat(sig_k):
        sr = np.sqrt(sig_k * sig_k - SIG_V * SIG_V)
        A = DG * sig_k / (np.sqrt(2 * np.pi) * SIG_V * sr)
        return A * np.exp(
            -((refd[None, :] - grid[:, None]) ** 2) / (2 * sr * sr)
        )

    kp_s = kp_mat(sig_s)  # [G, R]
    kp_c = kp_mat(sig_c)

    nc = _get_prog()

    t = S[:, 0].astype(np.float64)
    v = S[:, 1].astype(np.float64)
    dims = S[:, 2].astype(np.int32)
    mask = (t > 0).astype(np.float64)
    cnt = np.bincount(dims, minlength=NI).astype(np.float64)
    sv = np.bincount(dims, weights=v * mask, minlength=NI)
    corr = np.concatenate([EPS * (cnt + 1.0), EPS * sv]).astype(np.float32)
    corr = corr.reshape(P, 1)

    # host-precomputed stationary weights [N, 128] in bf16
    comb = np.zeros((N, 2 * NI), np.float32)
    rows = np.arange(N)
    comb[rows, dims] = mask
    comb[rows, NI + dims] = mask * v
    comb = comb.astype(ml_dtypes.bfloat16)

    gb = np.ascontiguousarray(
        np.broadcast_to(grid.astype(np.float32)[None, :], (P, G))
    )

    in_maps = []
    for i in range(M):
        comb_i = comb[i * ND : (i + 1) * ND].reshape(NCHUNK, P, 2 * NI)
        comb_i = np.ascontiguousarray(comb_i.transpose(1, 0, 2))
        t_i = (
            S[i * ND : (i + 1) * ND, 0]
            .reshape(NCHUNK, P)
            .transpose(1, 0)
        )
        kp_i = np.stack(
            [
                kp_s[:, i * RS : (i + 1) * RS],
                kp_c[:, i * RS : (i + 1) * RS],
            ],
            axis=1,
        ).astype(ml_dtypes.bfloat16)
        in_maps.append(
            {
                "comb": comb_i,
                "gb": gb,
                "tsc": np.ascontiguousarray(t_i.astype(np.float32)),
                "rho": rho,
                "corr": corr,
                "kp": np.ascontiguousarray(kp_i),
            }
        )

    if os.environ.get("BASS_SIM"):
        import concourse.libnrt as libnrt
        from concourse.bass_interp import MultiCoreSim

        # fake_nrt has no driver NC-map ioctls; identity maps match the
        # sim's 8-cores-on-one-device model
        libnrt.get_trn2_nc_mapping = lambda: {
            (d, i): i for d in range(4) for i in range(8)
        }
        libnrt.get_device_id_to_routing_id_mapping = lambda: {
            d: d for d in range(4)
        }
        import concourse.bass_interp as _bi

        _bi.get_device_id_to_routing_id_mapping = (
            libnrt.get_device_id_to_routing_id_mapping
        )

        sim = MultiCoreSim(nc, M)
        for i in range(M):
            for k, val in in_maps[i].items():
                sim.cores[i].tensor(k)[:] = val
        sim.simulate()
        out = np.concatenate(
            [
                np.transpose(
                    np.array(sim.cores[i].tensor("out")), (2, 1, 0)
                ).reshape(RS, 3 * NI)
                for i in range(M)
            ],
            axis=0,
        )
        last_results = None
    else:
        from concourse.bass_utils import run_bass_kernel_spmd

        tc_env = os.environ.get("BASS_TRACE_CORES")
        res = run_bass_kernel_spmd(
            nc,
            in_maps,
            list(range(M)),
            trace=bool(os.environ.get("BASS_TRACE")),
            trace_cores=(
                [int(x) for x in tc_env.split(",")] if tc_env else None
            ),
        )
        last_results = res
        out = np.concatenate(
            [
                np.transpose(
                    np.asarray(res.results[i]["out"]), (2, 1, 0)
                ).reshape(RS, 3 * NI)
                for i in range(M)
            ],
            axis=0,
        )

    return np.ascontiguousarray(out).reshape(1, R, 3 * NI).astype(np.float32)
